# revision 5
# baseline (speedup 1.0000x reference)
"""nn_Loss_20212116095273 Trainium2 Bass kernel.

Output: 0.99 * smooth_l1_map([16,200000]) + 0.01 * scalar_direction_loss.

Design (pedestrian-axis sharding, 8 NeuronCores, full inputs in / full output
out):
  * Each core gets a 25000-pedestrian shard laid out as 125 SBUF partitions
    x 200 peds, streamed in 5 frame-blocks (targets frames t..t+3).
  * Direction loss per (frame, ped): the five reference points' angles are
    computed via the cross/dot formulation theta = pi/2 - atan(D/|X|) with
    X = cross, D = dot of (pred-last, true-last) corner diff vectors; atan on
    ScalarE (full-range saturation verified on HW), 1/|X| = Rsqrt(X^2+eps) on
    ScalarE, everything else on VectorE in bf16 (scale-invariant math).
  * Smooth-L1 uses sl1(d) = m*(|d| - 0.5m), m = min(|d|,1), fused via
    scalar_tensor_tensor, channel-sum by plane adds.
  * Per-core angle sums come from free Arctan accum_out on ScalarE; a
    GPSIMD partition_all_reduce + an AllReduce collective across the 8 cores
    produce the global direction scalar, which is folded into the map as a
    per-partition bias on the final pass.
"""

import os
import sys

os.environ.setdefault("JAX_PLATFORMS", "")
if "/opt/trn_rl_repo" not in sys.path:
    sys.path.insert(0, "/opt/trn_rl_repo")

import numpy as np

import concourse.bacc as bacc
import concourse.mybir as mybir
import concourse.tile as tile
from concourse import bass_utils
from concourse.bass import AP
from concourse import bass_isa

AF = mybir.ActivationFunctionType
A = mybir.AluOpType
F32 = mybir.dt.float32
BF16 = mybir.dt.bfloat16

T = 16
F_DIR = 15
P_FULL = 200_000
N_CORES = 8


def _act_raw(nc, out, in_, func, bias=0.0, scale=1.0, accum_out=None):
    """InstActivation emission that allows Rsqrt (bass bans it by policy; the
    ~5e-5 rel accuracy is plenty here) and supports accum_out."""
    inputs = [nc.scalar.lower_ap(in_)]
    if func not in (AF.Copy, AF.Reciprocal):
        if isinstance(bias, float):
            bias = nc.const_aps.scalar_like(bias, in_)
    for arg in (bias, scale, 0.0):
        if isinstance(arg, AP):
            inputs.append(nc.scalar.lower_ap(arg))
        else:
            inputs.append(
                mybir.ImmediateValue(dtype=mybir.dt.float32, value=float(arg))
            )
    outputs = [nc.scalar.lower_ap(out)]
    if accum_out is not None:
        outputs.append(nc.scalar.lower_ap(accum_out))
    return nc.scalar.add_instruction(
        mybir.InstActivation(
            name=nc.get_next_instruction_name(), func=func, ins=inputs, outs=outputs
        )
    )


def build_loss_program(part, w, n_cores=N_CORES, p_full=None):
    """Build the per-core Bass program. Shard = part*w pedestrians."""
    p_shard = part * w
    if p_full is None:
        p_full = p_shard * n_cores
    sl1_scale = 0.99 / p_full
    # bias = 0.002*(2.5*pi - AR_total/(p_full*F_DIR))
    c_mul = -0.002 / (p_full * F_DIR)
    c_add = 0.002 * 2.5 * np.pi

    nc = bacc.Bacc(
        "TRN2",
        target_bir_lowering=False,
        debug=False,
        enable_asserts=True,
        num_devices=n_cores,
    )
    tgt_d = nc.dram_tensor("targets", [T, p_shard, 8], F32, kind="ExternalInput").ap()
    out_d = nc.dram_tensor("outputs", [T, p_shard, 4], F32, kind="ExternalInput").ap()
    res_d = nc.dram_tensor("out", [T, p_shard], F32, kind="ExternalOutput").ap()
    dir_d = nc.dram_tensor("dir_sum", [1, 1], F32, kind="ExternalOutput").ap()

    # (t0, nd, ns): dir frames t0..t0+nd-1, sl1 frames t0..t0+ns-1;
    # targets tile always holds frames t0..t0+3.
    blocks = [(0, 3, 3), (3, 3, 3), (6, 3, 3), (9, 3, 3), (12, 3, 4)]

    with tile.TileContext(nc) as tc:
        with (
            tc.tile_pool(name="io", bufs=2) as pio,
            tc.tile_pool(name="sc", bufs=1) as psc,
            tc.tile_pool(name="dram", bufs=1, space="DRAM") as pdram,
        ):
            # persistent tiles
            v_all = psc.tile([part, 5, F_DIR, w], BF16, name="v_all")
            accs = psc.tile([part, 8], F32, name="accs")
            rsb = psc.tile([part, 1], F32, name="rsb")
            nc.vector.memset(rsb[:], 1e-20)
            mapd = pdram.tile([part, T, w], F32, name="mapd")

            for bi, (t0, nd, ns) in enumerate(blocks):
                tgt = pio.tile([part, 4, w, 8], F32, tag="tgt", bufs=2)
                nc.sync.dma_start(
                    out=tgt[:],
                    in_=tgt_d[t0 : t0 + 4].rearrange("t (q w) c -> q t w c", q=part),
                )
                outp = pio.tile([part, ns, w, 4], F32, tag="outp", bufs=2)
                nc.sync.dma_start(
                    out=outp[:],
                    in_=out_d[t0 : t0 + ns].rearrange("t (q w) c -> q t w c", q=part),
                )

                la = tgt[:, 0:nd, :, 0]
                lb = tgt[:, 0:nd, :, 1]
                lc = tgt[:, 0:nd, :, 2]
                ld = tgt[:, 0:nd, :, 3]
                na = tgt[:, 1 : nd + 1, :, 0]
                nb = tgt[:, 1 : nd + 1, :, 1]
                nc_ = tgt[:, 1 : nd + 1, :, 2]
                nd_ = tgt[:, 1 : nd + 1, :, 3]
                oa = outp[:, 0:nd, :, 0]
                ob = outp[:, 0:nd, :, 1]
                oc = outp[:, 0:nd, :, 2]
                od = outp[:, 0:nd, :, 3]

                # ---- diffs (all values are 2x the true diffs; cos-invariant)
                u1 = psc.tile([part, nd, w], BF16, tag="u1")
                u2 = psc.tile([part, nd, w], BF16, tag="u2")
                v1 = psc.tile([part, nd, w], BF16, tag="v1")
                v2 = psc.tile([part, nd, w], BF16, tag="v2")
                v3 = psc.tile([part, nd, w], BF16, tag="v3")
                v4 = psc.tile([part, nd, w], BF16, tag="v4")
                D4 = psc.tile([part, 4, nd, w], BF16, tag="D4")
                E4 = psc.tile([part, 4, nd, w], BF16, tag="E4")

                stt = nc.vector.scalar_tensor_tensor
                tt = nc.vector.tensor_tensor
                ts = nc.vector.tensor_scalar

                stt(out=u1[:], in0=oa, scalar=2.0, in1=oc, op0=A.mult, op1=A.subtract)
                stt(out=u2[:], in0=ob, scalar=2.0, in1=od, op0=A.mult, op1=A.subtract)
                tt(out=D4[:, 0], in0=la, in1=u1[:], op=A.add)
                stt(out=D4[:, 1], in0=oa, scalar=2.0, in1=lc, op0=A.mult, op1=A.add)
                tt(out=D4[:, 2], in0=lb, in1=u2[:], op=A.add)
                stt(out=D4[:, 3], in0=ob, scalar=2.0, in1=ld, op0=A.mult, op1=A.add)
                tt(out=v1[:], in0=na, in1=la, op=A.subtract)
                tt(out=v3[:], in0=nb, in1=lb, op=A.subtract)
                stt(out=E4[:, 1], in0=v1[:], scalar=2.0, in1=lc, op0=A.mult, op1=A.add)
                stt(out=E4[:, 3], in0=v3[:], scalar=2.0, in1=ld, op0=A.mult, op1=A.add)
                tt(out=v2[:], in0=nc_, in1=la, op=A.subtract)
                tt(out=v4[:], in0=nd_, in1=lb, op=A.subtract)
                tt(out=E4[:, 0], in0=E4[:, 1], in1=v2[:], op=A.subtract)
                tt(out=E4[:, 2], in0=E4[:, 3], in1=v4[:], op=A.subtract)

                if t0 == 0:
                    # frame 0: "last" gets a single convert_bbox, not two.
                    w1 = psc.tile([part, 1, w], BF16, tag="w1")
                    w2 = psc.tile([part, 1, w], BF16, tag="w2")
                    f0 = slice(0, 1)
                    nc.vector.tensor_copy(out=D4[:, 0, 0], in_=u1[:, 0])
                    ts(out=D4[:, 1, 0], in0=oa[:, 0], scalar1=2.0, scalar2=None, op0=A.mult)
                    nc.vector.tensor_copy(out=D4[:, 2, 0], in_=u2[:, 0])
                    ts(out=D4[:, 3, 0], in0=ob[:, 0], scalar1=2.0, scalar2=None, op0=A.mult)
                    tt(out=w1[:, 0], in0=nc_[:, 0], in1=lc[:, 0], op=A.subtract)
                    tt(out=w2[:, 0], in0=nd_[:, 0], in1=ld[:, 0], op=A.subtract)
                    ts(out=E4[:, 1, 0], in0=v1[:, 0], scalar1=2.0, scalar2=None, op0=A.mult)
                    ts(out=E4[:, 3, 0], in0=v3[:, 0], scalar1=2.0, scalar2=None, op0=A.mult)
                    stt(out=E4[:, 0, 0], in0=v1[:, 0], scalar=2.0, in1=w1[:, 0],
                        op0=A.mult, op1=A.subtract)
                    stt(out=E4[:, 2, 0], in0=v3[:, 0], scalar=2.0, in1=w2[:, 0],
                        op0=A.mult, op1=A.subtract)

                # ---- bilinear: dots and crosses for 5 points
                AB = psc.tile([part, 4, nd, w], BF16, tag="AB")
                X5 = psc.tile([part, 5, nd, w], BF16, tag="X5")
                D5 = psc.tile([part, 5, nd, w], BF16, tag="D5")
                tt(out=AB[:, 0], in0=D4[:, 0], in1=E4[:, 0], op=A.mult)
                tt(out=AB[:, 1], in0=D4[:, 1], in1=E4[:, 1], op=A.mult)
                tt(out=AB[:, 2], in0=D4[:, 2], in1=E4[:, 2], op=A.mult)
                tt(out=AB[:, 3], in0=D4[:, 3], in1=E4[:, 3], op=A.mult)
                # corners in plane order (i,j) = (0,0),(0,1),(1,0),(1,1)
                pt = psc.tile([part, nd, w], BF16, tag="pt")
                qt = psc.tile([part, nd, w], BF16, tag="qt")
                for k, (i, j) in enumerate([(0, 0), (0, 1), (1, 0), (1, 1)]):
                    tt(out=pt[:], in0=D4[:, i], in1=E4[:, 2 + j], op=A.mult)
                    tt(out=qt[:], in0=D4[:, 2 + j], in1=E4[:, i], op=A.mult)
                    tt(out=X5[:, 1 + k], in0=pt[:], in1=qt[:], op=A.subtract)
                    tt(out=D5[:, 1 + k], in0=AB[:, i], in1=AB[:, 2 + j], op=A.add)
                # center: cross = sum of corner crosses; dot from summed vecs
                S2 = psc.tile([part, 2, nd, w], BF16, tag="S2")
                T2 = psc.tile([part, 2, nd, w], BF16, tag="T2")
                M2 = psc.tile([part, 2, nd, w], BF16, tag="M2")
                tt(out=S2[:, 0], in0=D4[:, 0], in1=D4[:, 1], op=A.add)
                tt(out=S2[:, 1], in0=D4[:, 2], in1=D4[:, 3], op=A.add)
                tt(out=T2[:, 0], in0=E4[:, 0], in1=E4[:, 1], op=A.add)
                tt(out=T2[:, 1], in0=E4[:, 2], in1=E4[:, 3], op=A.add)
                tt(out=M2[:], in0=S2[:], in1=T2[:], op=A.mult)
                tt(out=D5[:, 0], in0=M2[:, 0], in1=M2[:, 1], op=A.add)
                Xu = psc.tile([part, 2, nd, w], BF16, tag="Xu")
                tt(out=Xu[:], in0=X5[:, 1:3], in1=X5[:, 3:5], op=A.add)
                tt(out=X5[:, 0], in0=Xu[:, 0], in1=Xu[:, 1], op=A.add)

                # ---- angle argument v = D / |X| = D * rsqrt(X^2 + eps)
                # X5 is squared in place, then rsqrt'd in place.
                tt(out=X5[:], in0=X5[:], in1=X5[:], op=A.mult)
                _act_raw(nc, X5[:], X5[:], AF.Rsqrt, bias=rsb[:])
                tt(out=v_all[:, :, t0 : t0 + nd, :], in0=D5[:], in1=X5[:], op=A.mult)

                # ---- smooth l1 for frames t0..t0+ns-1
                dp = psc.tile([part, 4, ns, w], BF16, tag="dp")
                mt = psc.tile([part, 4, ns, w], BF16, tag="mt")
                sut = psc.tile([part, 2, ns, w], BF16, tag="sut")
                mapb = pio.tile([part, ns, w], F32, tag="mapb", bufs=2)
                dp_planar = dp[:].rearrange("p c t w -> p t w c")
                tt(out=dp_planar, in0=outp[:, 0:ns, :, :],
                   in1=tgt[:, 0:ns, :, 4:8], op=A.subtract)
                # |d| in place over d, then m = min(|d|, 1)
                adt = dp
                stt(out=adt[:], in0=dp[:], scalar=-1.0, in1=dp[:], op0=A.mult, op1=A.max)
                ts(out=mt[:], in0=adt[:], scalar1=1.0, scalar2=None, op0=A.min)
                # z = ad - 0.5 m (in place over ad), sl = (m*scale)*z (over m)
                stt(out=adt[:], in0=mt[:], scalar=-0.5, in1=adt[:], op0=A.mult, op1=A.add)
                stt(out=mt[:], in0=mt[:], scalar=sl1_scale, in1=adt[:], op0=A.mult, op1=A.mult)
                tt(out=sut[:], in0=mt[:, 0:2], in1=mt[:, 2:4], op=A.add)
                tt(out=mapb[:], in0=sut[:, 0], in1=sut[:, 1], op=A.add)
                nc.sync.dma_start(out=mapd[:, t0 : t0 + ns, :], in_=mapb[:])

            # ---- direction-loss reduction
            atsc = psc.tile([part, F_DIR * w], BF16, name="atsc")
            for k in range(5):
                _act_raw(
                    nc,
                    atsc[:],
                    v_all[:, k].rearrange("p t w -> p (t w)"),
                    AF.Arctan,
                    accum_out=accs[:, k : k + 1],
                )
            dirp = psc.tile([part, 1], F32, name="dirp")
            nc.vector.tensor_reduce(
                out=dirp[:], in_=accs[:, 0:5], axis=mybir.AxisListType.X, op=A.add
            )
            arp = psc.tile([part, 1], F32, name="arp")
            nc.gpsimd.partition_all_reduce(
                arp[:], dirp[:], part, bass_isa.ReduceOp.add
            )
            cin = pdram.tile([1, 1], F32, name="cin")
            cout = pdram.tile([1, 1], F32, name="cout")
            nc.sync.dma_start(out=cin[:], in_=arp[0:1, :])
            nc.gpsimd.collective_compute(
                "AllReduce",
                A.add,
                replica_groups=[list(range(n_cores))],
                ins=[cin.opt()],
                outs=[cout.opt()],
            )
            allr = psc.tile([1, 1], F32, name="allr")
            nc.sync.dma_start(out=allr[:], in_=cout[:])
            nc.sync.dma_start(out=dir_d[:], in_=cout[:])
            bb = psc.tile([part, 1], F32, name="bb")
            nc.gpsimd.partition_broadcast(bb[:], allr[:])
            bias_t = psc.tile([part, 1], F32, name="bias_t")
            nc.vector.tensor_scalar(
                out=bias_t[:], in0=bb[:], scalar1=c_mul, scalar2=c_add,
                op0=A.mult, op1=A.add,
            )

            # ---- final: out = map + bias (in place)
            mapin = psc.tile([part, T, w], F32, name="mapin")
            nc.sync.dma_start(out=mapin[:], in_=mapd[:])
            nc.vector.tensor_scalar(
                out=mapin[:], in0=mapin[:], scalar1=bias_t[:], scalar2=None, op0=A.add
            )
            nc.sync.dma_start(
                out=res_d.rearrange("t (q w) -> q t w", q=part), in_=mapin[:]
            )

    nc.compile()
    return nc


_CACHE = {}


def _get_program(part, w):
    key = (part, w)
    if key not in _CACHE:
        _CACHE[key] = build_loss_program(part, w)
    return _CACHE[key]


def kernel(outputs: np.ndarray, targets: np.ndarray) -> np.ndarray:
    outputs = np.ascontiguousarray(np.asarray(outputs, dtype=np.float32))
    targets = np.ascontiguousarray(np.asarray(targets, dtype=np.float32))
    assert outputs.shape == (T, P_FULL, 4) and targets.shape == (T, P_FULL, 8)

    part, w = 125, 200
    p_shard = part * w
    nc = _get_program(part, w)

    in_maps = []
    for c in range(N_CORES):
        sl = slice(c * p_shard, (c + 1) * p_shard)
        in_maps.append(
            {
                "targets": np.ascontiguousarray(targets[:, sl, :]),
                "outputs": np.ascontiguousarray(outputs[:, sl, :]),
            }
        )
    res = bass_utils.run_bass_kernel_spmd(nc, in_maps, core_ids=list(range(N_CORES)))
    out = np.concatenate([res.results[c]["out"] for c in range(N_CORES)], axis=1)
    return out.astype(np.float32)


# revision 9
# speedup vs baseline: 1.3465x; 1.3465x over previous
"""nn_Loss_20212116095273 Trainium2 Bass kernel.

Output: 0.99 * smooth_l1_map([16,200000]) + 0.01 * scalar_direction_loss.

Design (pedestrian-axis sharding, 8 NeuronCores, full inputs in / full output
out):
  * Each core gets a 25000-pedestrian shard laid out as 125 SBUF partitions
    x 200 peds, streamed in 5 frame-blocks (targets frames t..t+3).
  * Direction loss per (frame, ped): the five reference points' angles are
    computed via the cross/dot formulation theta = pi/2 - atan(D/|X|) with
    X = cross, D = dot of (pred-last, true-last) corner diff vectors; atan on
    ScalarE (full-range saturation verified on HW), 1/|X| = Rsqrt(X^2+eps) on
    ScalarE, everything else on VectorE in bf16 (scale-invariant math).
  * Smooth-L1 uses sl1(d) = m*(|d| - 0.5m), m = min(|d|,1), fused via
    scalar_tensor_tensor, channel-sum by plane adds.
  * Per-core angle sums come from free Arctan accum_out on ScalarE; a
    GPSIMD partition_all_reduce + an AllReduce collective across the 8 cores
    produce the global direction scalar, which is folded into the map as a
    per-partition bias on the final pass.
"""

import os
import sys

os.environ.setdefault("JAX_PLATFORMS", "")
if "/opt/trn_rl_repo" not in sys.path:
    sys.path.insert(0, "/opt/trn_rl_repo")

import numpy as np

import concourse.bacc as bacc
import concourse.mybir as mybir
import concourse.tile as tile
from concourse import bass_utils
from concourse.bass import AP
from concourse import bass_isa

AF = mybir.ActivationFunctionType
A = mybir.AluOpType
F32 = mybir.dt.float32
BF16 = mybir.dt.bfloat16

T = 16
F_DIR = 15
P_FULL = 200_000
N_CORES = 8


def _act_raw(nc, out, in_, func, bias=0.0, scale=1.0, accum_out=None):
    """InstActivation emission that allows Rsqrt (bass bans it by policy; the
    ~5e-5 rel accuracy is plenty here) and supports accum_out."""
    inputs = [nc.scalar.lower_ap(in_)]
    if func not in (AF.Copy, AF.Reciprocal):
        if isinstance(bias, float):
            bias = nc.const_aps.scalar_like(bias, in_)
    for arg in (bias, scale, 0.0):
        if isinstance(arg, AP):
            inputs.append(nc.scalar.lower_ap(arg))
        else:
            inputs.append(
                mybir.ImmediateValue(dtype=mybir.dt.float32, value=float(arg))
            )
    outputs = [nc.scalar.lower_ap(out)]
    if accum_out is not None:
        outputs.append(nc.scalar.lower_ap(accum_out))
    return nc.scalar.add_instruction(
        mybir.InstActivation(
            name=nc.get_next_instruction_name(), func=func, ins=inputs, outs=outputs
        )
    )


def build_loss_program(part, w, n_cores=N_CORES, p_full=None, collective=True):
    """Build the per-core Bass program. Shard = part*w pedestrians."""
    p_shard = part * w
    if p_full is None:
        p_full = p_shard * n_cores
    sl1_scale = 0.99 / p_full
    # bias = 0.002*(2.5*pi - AR_total/(p_full*F_DIR))
    c_mul = -0.002 / (p_full * F_DIR)
    c_add = 0.002 * 2.5 * np.pi

    nc = bacc.Bacc(
        "TRN2",
        target_bir_lowering=False,
        debug=False,
        enable_asserts=True,
        num_devices=n_cores,
    )
    tgt_d = nc.dram_tensor("targets", [T, p_shard, 8], F32, kind="ExternalInput").ap()
    out_d = nc.dram_tensor("outputs", [T, p_shard, 4], F32, kind="ExternalInput").ap()
    res_d = nc.dram_tensor("out", [T, p_shard], F32, kind="ExternalOutput").ap()
    dir_d = nc.dram_tensor("dir_sum", [1, 1], F32, kind="ExternalOutput").ap()

    # (t0, nd, ns): dir frames t0..t0+nd-1, sl1 frames t0..t0+ns-1;
    # targets tile always holds frames t0..t0+3.
    blocks = [(0, 3, 3), (3, 3, 3), (6, 3, 3), (9, 3, 3), (12, 3, 4)]

    with tile.TileContext(nc) as tc:
        with (
            tc.tile_pool(name="io", bufs=2) as pio,
            tc.tile_pool(name="sc", bufs=1) as psc,
            tc.tile_pool(name="dram", bufs=1, space="DRAM") as pdram,
        ):
            # persistent tiles
            v_all = psc.tile([part, 5, F_DIR, w], BF16, name="v_all")
            accs = psc.tile([part, 8], F32, name="accs")
            rsb = psc.tile([part, 1], F32, name="rsb")
            nc.vector.memset(rsb[:], 1e-20)
            mapd = pdram.tile([part, T, w], F32, name="mapd")

            for bi, (t0, nd, ns) in enumerate(blocks):
                tgt = pio.tile([part, 4, w, 8], F32, tag="tgt", bufs=2)
                nc.sync.dma_start(
                    out=tgt[:],
                    in_=tgt_d[t0 : t0 + 4].rearrange("t (q w) c -> q t w c", q=part),
                )
                outp = pio.tile([part, ns, w, 4], F32, tag="outp", bufs=2)
                nc.sync.dma_start(
                    out=outp[:],
                    in_=out_d[t0 : t0 + ns].rearrange("t (q w) c -> q t w c", q=part),
                )

                la = tgt[:, 0:nd, :, 0]
                lb = tgt[:, 0:nd, :, 1]
                lc = tgt[:, 0:nd, :, 2]
                ld = tgt[:, 0:nd, :, 3]
                na = tgt[:, 1 : nd + 1, :, 0]
                nb = tgt[:, 1 : nd + 1, :, 1]
                nc_ = tgt[:, 1 : nd + 1, :, 2]
                nd_ = tgt[:, 1 : nd + 1, :, 3]
                oa = outp[:, 0:nd, :, 0]
                ob = outp[:, 0:nd, :, 1]
                oc = outp[:, 0:nd, :, 2]
                od = outp[:, 0:nd, :, 3]

                # ---- diffs (all values are 2x the true diffs; cos-invariant)
                u1 = psc.tile([part, nd, w], BF16, tag="u1")
                u2 = psc.tile([part, nd, w], BF16, tag="u2")
                v1 = psc.tile([part, nd, w], BF16, tag="v1")
                v2 = psc.tile([part, nd, w], BF16, tag="v2")
                v3 = psc.tile([part, nd, w], BF16, tag="v3")
                v4 = psc.tile([part, nd, w], BF16, tag="v4")
                D4 = psc.tile([part, 4, nd, w], BF16, tag="D4")
                E4 = psc.tile([part, 4, nd, w], BF16, tag="E4")

                stt = nc.vector.scalar_tensor_tensor
                tt = nc.vector.tensor_tensor
                ts = nc.vector.tensor_scalar

                stt(out=u1[:], in0=oa, scalar=2.0, in1=oc, op0=A.mult, op1=A.subtract)
                stt(out=u2[:], in0=ob, scalar=2.0, in1=od, op0=A.mult, op1=A.subtract)
                tt(out=D4[:, 0], in0=la, in1=u1[:], op=A.add)
                stt(out=D4[:, 1], in0=oa, scalar=2.0, in1=lc, op0=A.mult, op1=A.add)
                tt(out=D4[:, 2], in0=lb, in1=u2[:], op=A.add)
                stt(out=D4[:, 3], in0=ob, scalar=2.0, in1=ld, op0=A.mult, op1=A.add)
                tt(out=v1[:], in0=na, in1=la, op=A.subtract)
                tt(out=v3[:], in0=nb, in1=lb, op=A.subtract)
                stt(out=E4[:, 1], in0=v1[:], scalar=2.0, in1=lc, op0=A.mult, op1=A.add)
                stt(out=E4[:, 3], in0=v3[:], scalar=2.0, in1=ld, op0=A.mult, op1=A.add)
                tt(out=v2[:], in0=nc_, in1=la, op=A.subtract)
                tt(out=v4[:], in0=nd_, in1=lb, op=A.subtract)
                tt(out=E4[:, 0], in0=E4[:, 1], in1=v2[:], op=A.subtract)
                tt(out=E4[:, 2], in0=E4[:, 3], in1=v4[:], op=A.subtract)

                if t0 == 0:
                    # frame 0: "last" gets a single convert_bbox, not two.
                    w1 = psc.tile([part, 1, w], BF16, tag="w1")
                    w2 = psc.tile([part, 1, w], BF16, tag="w2")
                    f0 = slice(0, 1)
                    nc.vector.tensor_copy(out=D4[:, 0, 0], in_=u1[:, 0])
                    ts(out=D4[:, 1, 0], in0=oa[:, 0], scalar1=2.0, scalar2=None, op0=A.mult)
                    nc.vector.tensor_copy(out=D4[:, 2, 0], in_=u2[:, 0])
                    ts(out=D4[:, 3, 0], in0=ob[:, 0], scalar1=2.0, scalar2=None, op0=A.mult)
                    tt(out=w1[:, 0], in0=nc_[:, 0], in1=lc[:, 0], op=A.subtract)
                    tt(out=w2[:, 0], in0=nd_[:, 0], in1=ld[:, 0], op=A.subtract)
                    ts(out=E4[:, 1, 0], in0=v1[:, 0], scalar1=2.0, scalar2=None, op0=A.mult)
                    ts(out=E4[:, 3, 0], in0=v3[:, 0], scalar1=2.0, scalar2=None, op0=A.mult)
                    stt(out=E4[:, 0, 0], in0=v1[:, 0], scalar=2.0, in1=w1[:, 0],
                        op0=A.mult, op1=A.subtract)
                    stt(out=E4[:, 2, 0], in0=v3[:, 0], scalar=2.0, in1=w2[:, 0],
                        op0=A.mult, op1=A.subtract)

                # ---- bilinear: dots and crosses for 5 points
                AB = psc.tile([part, 4, nd, w], BF16, tag="AB")
                X5 = psc.tile([part, 5, nd, w], BF16, tag="X5")
                D5 = psc.tile([part, 5, nd, w], BF16, tag="D5")
                tt(out=AB[:, 0], in0=D4[:, 0], in1=E4[:, 0], op=A.mult)
                tt(out=AB[:, 1], in0=D4[:, 1], in1=E4[:, 1], op=A.mult)
                tt(out=AB[:, 2], in0=D4[:, 2], in1=E4[:, 2], op=A.mult)
                tt(out=AB[:, 3], in0=D4[:, 3], in1=E4[:, 3], op=A.mult)
                # corners in plane order (i,j) = (0,0),(0,1),(1,0),(1,1)
                pt = psc.tile([part, nd, w], BF16, tag="pt")
                qt = psc.tile([part, nd, w], BF16, tag="qt")
                for k, (i, j) in enumerate([(0, 0), (0, 1), (1, 0), (1, 1)]):
                    tt(out=pt[:], in0=D4[:, i], in1=E4[:, 2 + j], op=A.mult)
                    tt(out=qt[:], in0=D4[:, 2 + j], in1=E4[:, i], op=A.mult)
                    tt(out=X5[:, 1 + k], in0=pt[:], in1=qt[:], op=A.subtract)
                    tt(out=D5[:, 1 + k], in0=AB[:, i], in1=AB[:, 2 + j], op=A.add)
                # center: cross = sum of corner crosses; dot from summed vecs
                S2 = psc.tile([part, 2, nd, w], BF16, tag="S2")
                T2 = psc.tile([part, 2, nd, w], BF16, tag="T2")
                M2 = psc.tile([part, 2, nd, w], BF16, tag="M2")
                tt(out=S2[:, 0], in0=D4[:, 0], in1=D4[:, 1], op=A.add)
                tt(out=S2[:, 1], in0=D4[:, 2], in1=D4[:, 3], op=A.add)
                tt(out=T2[:, 0], in0=E4[:, 0], in1=E4[:, 1], op=A.add)
                tt(out=T2[:, 1], in0=E4[:, 2], in1=E4[:, 3], op=A.add)
                tt(out=M2[:], in0=S2[:], in1=T2[:], op=A.mult)
                tt(out=D5[:, 0], in0=M2[:, 0], in1=M2[:, 1], op=A.add)
                Xu = psc.tile([part, 2, nd, w], BF16, tag="Xu")
                tt(out=Xu[:], in0=X5[:, 1:3], in1=X5[:, 3:5], op=A.add)
                tt(out=X5[:, 0], in0=Xu[:, 0], in1=Xu[:, 1], op=A.add)

                # ---- angle argument v = D / |X| = D * rsqrt(X^2 + eps)
                # X5 is squared in place, then rsqrt'd in place.
                tt(out=X5[:], in0=X5[:], in1=X5[:], op=A.mult)
                _act_raw(nc, X5[:], X5[:], AF.Rsqrt, bias=rsb[:])
                tt(out=v_all[:, :, t0 : t0 + nd, :], in0=D5[:], in1=X5[:], op=A.mult)

                # ---- smooth l1 for frames t0..t0+ns-1
                dp = psc.tile([part, 4, ns, w], BF16, tag="dp")
                mt = psc.tile([part, 4, ns, w], BF16, tag="mt")
                sut = psc.tile([part, 2, ns, w], BF16, tag="sut")
                mapb = pio.tile([part, ns, w], F32, tag="mapb", bufs=2)
                dp_planar = dp[:].rearrange("p c t w -> p t w c")
                tt(out=dp_planar, in0=outp[:, 0:ns, :, :],
                   in1=tgt[:, 0:ns, :, 4:8], op=A.subtract)
                # |d| in place over d, then m = min(|d|, 1)
                adt = dp
                stt(out=adt[:], in0=dp[:], scalar=-1.0, in1=dp[:], op0=A.mult, op1=A.max)
                ts(out=mt[:], in0=adt[:], scalar1=1.0, scalar2=None, op0=A.min)
                # z = ad - 0.5 m (in place over ad), sl = (m*scale)*z (over m)
                stt(out=adt[:], in0=mt[:], scalar=-0.5, in1=adt[:], op0=A.mult, op1=A.add)
                stt(out=mt[:], in0=mt[:], scalar=sl1_scale, in1=adt[:], op0=A.mult, op1=A.mult)
                tt(out=sut[:], in0=mt[:, 0:2], in1=mt[:, 2:4], op=A.add)
                tt(out=mapb[:], in0=sut[:, 0], in1=sut[:, 1], op=A.add)
                nc.sync.dma_start(out=mapd[:, t0 : t0 + ns, :], in_=mapb[:])

            # ---- direction-loss reduction
            atsc = psc.tile([part, F_DIR * w], BF16, name="atsc")
            for k in range(5):
                _act_raw(
                    nc,
                    atsc[:],
                    v_all[:, k].rearrange("p t w -> p (t w)"),
                    AF.Arctan,
                    accum_out=accs[:, k : k + 1],
                )
            dirp = psc.tile([part, 1], F32, name="dirp")
            nc.vector.tensor_reduce(
                out=dirp[:], in_=accs[:, 0:5], axis=mybir.AxisListType.X, op=A.add
            )
            arp = psc.tile([part, 1], F32, name="arp")
            nc.gpsimd.partition_all_reduce(
                arp[:], dirp[:], part, bass_isa.ReduceOp.add
            )
            cin = pdram.tile([1, 1], F32, name="cin")
            cout = pdram.tile([1, 1], F32, name="cout")
            nc.sync.dma_start(out=cin[:], in_=arp[0:1, :])
            if collective:
                nc.gpsimd.collective_compute(
                    "AllReduce",
                    A.add,
                    replica_groups=[list(range(n_cores))],
                    ins=[cin.opt()],
                    outs=[cout.opt()],
                )
            else:  # timeline-sim variant: no cross-core reduce
                nc.sync.dma_start(out=cout[:], in_=cin[:])
            allr = psc.tile([1, 1], F32, name="allr")
            nc.sync.dma_start(out=allr[:], in_=cout[:])
            nc.sync.dma_start(out=dir_d[:], in_=cout[:])
            bb = psc.tile([part, 1], F32, name="bb")
            nc.gpsimd.partition_broadcast(bb[:], allr[:])
            bias_t = psc.tile([part, 1], F32, name="bias_t")
            nc.vector.tensor_scalar(
                out=bias_t[:], in0=bb[:], scalar1=c_mul, scalar2=c_add,
                op0=A.mult, op1=A.add,
            )

            # ---- final: out = map + bias (in place)
            mapin = psc.tile([part, T, w], F32, name="mapin")
            nc.sync.dma_start(out=mapin[:], in_=mapd[:])
            nc.vector.tensor_scalar(
                out=mapin[:], in0=mapin[:], scalar1=bias_t[:], scalar2=None, op0=A.add
            )
            nc.sync.dma_start(
                out=res_d.rearrange("t (q w) -> q t w", q=part), in_=mapin[:]
            )

    nc.compile()
    return nc


_CACHE = {}
LAST_EXEC_NS = None


class _Runner:
    """Compile the Bass program once into a jitted shard_map executable.

    Mirrors bass2jax.run_bass_via_pjrt's multi-core branch, but caches the
    jitted callable so repeat kernel() calls skip retracing/recompiling.
    """

    def __init__(self, nc, n_cores):
        import jax
        from jax.sharding import Mesh, PartitionSpec
        from jax.experimental.shard_map import shard_map
        from concourse import bass2jax

        bass2jax.install_neuronx_cc_hook()
        assert nc.dbg_addr is None
        partition_name = (
            nc.partition_id_tensor.name if nc.partition_id_tensor else None
        )
        in_names, out_names, out_avals = [], [], []
        for alloc in nc.m.functions[0].allocations:
            if not isinstance(alloc, mybir.MemoryLocationSet):
                continue
            name = alloc.memorylocations[0].name
            if alloc.kind == "ExternalInput":
                if name != partition_name:
                    in_names.append(name)
            elif alloc.kind == "ExternalOutput":
                out_names.append(name)
                out_avals.append(
                    jax.core.ShapedArray(
                        tuple(alloc.tensor_shape), mybir.dt.np(alloc.dtype)
                    )
                )
        self.in_names, self.out_names, self.out_avals = in_names, out_names, out_avals
        self.n_cores = n_cores
        n_params = len(in_names)
        all_names = list(in_names) + list(out_names)
        if partition_name is not None:
            all_names.append(partition_name)
        all_names = tuple(all_names)
        donate = tuple(range(n_params, n_params + len(out_names)))

        def _body(*args):
            operands = list(args)
            if partition_name is not None:
                operands.append(bass2jax.partition_id_tensor())
            outs = bass2jax._bass_exec_p.bind(
                *operands,
                out_avals=tuple(out_avals),
                in_names=all_names,
                out_names=tuple(out_names),
                lowering_input_output_aliases=(),
                sim_require_finite=True,
                sim_require_nnan=True,
                nc=nc,
            )
            return tuple(outs)

        devices = jax.devices()[:n_cores]
        self.mesh = Mesh(np.asarray(devices), ("core",))
        self.pspec = PartitionSpec("core")
        n_io = n_params + len(out_names)
        self.sharded = jax.jit(
            shard_map(
                _body,
                mesh=self.mesh,
                in_specs=(self.pspec,) * n_io,
                out_specs=(self.pspec,) * len(out_names),
                check_rep=False,
            ),
            donate_argnums=donate,
            keep_unused=True,
        )

    def zero_outs(self):
        return [
            np.zeros((self.n_cores * a.shape[0], *a.shape[1:]), a.dtype)
            for a in self.out_avals
        ]

    def __call__(self, concat_inputs):
        args = [concat_inputs[n] for n in self.in_names]
        return self.sharded(*args, *self.zero_outs())


def _get_runner(part, w):
    key = (part, w)
    if key not in _CACHE:
        nc = build_loss_program(part, w)
        _CACHE[key] = _Runner(nc, N_CORES)
    return _CACHE[key]


def _shard_concat(x, part_peds):
    """[T, P, ...] -> [N_CORES*T, P/N_CORES, ...] (core-major concat)."""
    t = x.shape[0]
    rest = x.shape[2:]
    return np.ascontiguousarray(
        x.reshape(t, N_CORES, part_peds, *rest).transpose(1, 0, 2, *range(3, 3 + len(rest)))
        .reshape(N_CORES * t, part_peds, *rest)
    )


def kernel(outputs: np.ndarray, targets: np.ndarray) -> np.ndarray:
    global LAST_EXEC_NS
    import time
    import jax

    outputs = np.asarray(outputs, dtype=np.float32)
    targets = np.asarray(targets, dtype=np.float32)
    assert outputs.shape == (T, P_FULL, 4) and targets.shape == (T, P_FULL, 8)

    part, w = 125, 200
    p_shard = part * w
    runner = _get_runner(part, w)

    concat = {
        "targets": _shard_concat(targets, p_shard),
        "outputs": _shard_concat(outputs, p_shard),
    }
    t0 = time.perf_counter()
    out_arrs = runner(concat)
    jax.block_until_ready(out_arrs)
    LAST_EXEC_NS = int((time.perf_counter() - t0) * 1e9)
    by_name = dict(zip(runner.out_names, out_arrs))
    out = np.asarray(by_name["out"]).reshape(N_CORES, T, p_shard)
    return np.ascontiguousarray(out.transpose(1, 0, 2).reshape(T, P_FULL)).astype(
        np.float32
    )
